# revision 11
# baseline (speedup 1.0000x reference)
"""No-softmax attention Trainium2 kernel.

Math (per batch b, X = x[b] in [S, E], torch-Linear weights W[f, e]):
    Q = X Wq^T + bq ; K = X Wk^T + bk ; V = X Wv^T + bv
    y = (scale * Q K^T V) Wo^T + bo

No softmax => reassociate and fold all weights around the data Gram matrix:
    G  = X^T X                     [E, E]   (symmetric)
    s  = X^T 1                     [E]      (column sums)
    M  = K^T V = Wk G Wv^T + (Wk s) bv^T + bk (Wv s)^T + S bk bv^T
    y  = X A + 1 c^T
    A  = Wqs^T M Wo^T              (Wqs = scale*Wq folded on host)
    c  = bqs^T M Wo^T + bo         (bqs = scale*bq)

On-chip products (lhsT.T @ rhs with contraction on partitions; the only big
transposes are Wo and the output half of X, done on the PE):
    Ut  = Wk^T Wqs                 -> U^T
    T1t = G^T Ut = (U G)^T         (G symmetric)
    Rt  = Wv^T Wo^T                (uses WoT from PE transposes)
    A   = T1t^T Rt + u1 v1^T + u2 (v2 + S v1)^T   (rank-1 terms via padded
                                                   K=128 matmul)
    Y   = (X_half^T)^T A + 1 c^T   (uses XT from PE transposes)

Sharding: 8 cores = (batch b in 0..3) x (sequence half h in 0..1). Every core
computes the full per-batch G/A chain (duplicated across the pair) and its own
half of the output rows. Host permutes xb so rows 0..SH-1 are always the
core's half (G is row-order invariant).

Precision: matmuls run in float32r (fp22 inputs, fp32 accumulation, full PE
rate). DMA'd operands are round-to-nearest'ed to fp22 on the host so the
device-side fp22 read is lossless; intermediate products are rounded by the
DVE/ACT fp32->fp32r converting copies out of PSUM (~5e-4 rel overall).
"""

import numpy as np
from contextlib import ExitStack

import concourse.bass as bass
import concourse.tile as tile
from concourse import bacc, mybir
F32 = mybir.dt.float32
FR = mybir.dt.float32r
ALU = mybir.AluOpType

P = 128


def build_nc(S=2048, SH=1024, E=1024, num_devices=8):
    """Build the per-core SPMD program. All cores run the identical program."""
    NF = min(512, E)          # matmul moving free dim (fp32 PSUM bank limit)
    KO = S // P               # row chunks of full X
    SC = SH // P              # row chunks of the output half
    EC = E // P               # chunks of the embedding dim
    NT = E // NF              # free-dim tiles of E
    scaleS = float(S)         # the "S" in the rank-1 folds

    nc = bacc.Bacc("TRN2", target_bir_lowering=False, debug=False,
                   num_devices=num_devices)

    xb = nc.dram_tensor("xb", [S, E], FR, kind="ExternalInput").ap()
    wq = nc.dram_tensor("wq", [E, E], FR, kind="ExternalInput").ap()
    wk = nc.dram_tensor("wk", [E, E], FR, kind="ExternalInput").ap()
    wv = nc.dram_tensor("wv", [E, E], FR, kind="ExternalInput").ap()
    wo = nc.dram_tensor("wo", [E, E], FR, kind="ExternalInput").ap()
    bq = nc.dram_tensor("bq", [E], FR, kind="ExternalInput").ap()
    bk = nc.dram_tensor("bk", [E], FR, kind="ExternalInput").ap()
    bv = nc.dram_tensor("bv", [E], FR, kind="ExternalInput").ap()
    bo = nc.dram_tensor("bo", [E], FR, kind="ExternalInput").ap()
    idin = nc.dram_tensor("idin", [P, P], FR, kind="ExternalInput").ap()
    zin = nc.dram_tensor("zin", [P, E], FR, kind="ExternalInput").ap()
    augin = nc.dram_tensor("augin", [P, P], FR, kind="ExternalInput").ap()
    onein = nc.dram_tensor("onein", [P, 2], FR, kind="ExternalInput").ap()
    y = nc.dram_tensor("y", [SH, E], F32, kind="ExternalOutput").ap()

    with tile.TileContext(nc) as tc:
        _build(tc, locals())
    nc.compile()
    return nc


def _build(tc, t):
    nc = tc.nc
    S, SH, E, NF, KO, SC, EC, NT, scaleS = (
        t["S"], t["SH"], t["E"], t["NF"], t["KO"], t["SC"], t["EC"], t["NT"],
        t["scaleS"])
    xb, wq, wk, wv, wo, bq, bk, bv, bo, y, idin = (
        t["xb"], t["wq"], t["wk"], t["wv"], t["wo"], t["bq"], t["bk"],
        t["bv"], t["bo"], t["y"], t["idin"])
    zin, augin, onein = t["zin"], t["augin"], t["onein"]

    def mm(psum, lhsT, rhs, start, stop):
        nc.tensor.matmul(psum, lhsT, rhs, start=start, stop=stop)

    def rcopy(dst, src):
        # PSUM(fp32) -> SBUF(fp32r) converting copy; DVE rounds to fp22
        nc.vector.tensor_copy(dst, src)

    ctx = ExitStack()
    with ctx:
        consts = ctx.enter_context(tc.tile_pool(name="consts", bufs=1))
        psmm = ctx.enter_context(tc.tile_pool(name="psmm", bufs=3,
                                              space="PSUM"))
        pstr = ctx.enter_context(tc.tile_pool(name="pstr", bufs=2,
                                              space="PSUM"))
        psv = ctx.enter_context(tc.tile_pool(name="psv", bufs=2,
                                             space="PSUM"))
        dram = ctx.enter_context(tc.tile_pool(name="dram", bufs=1,
                                              space="DRAM"))
        stage = ctx.enter_context(tc.tile_pool(name="stage", bufs=3))

        ident = consts.tile([P, P], FR, tag="ident")
        nc.sync.dma_start(ident[:], idin[:])

        # constant small tensors
        # vector tiles get one zero pad column: fp32r matmuls need an even
        # moving free size, so vector operands are fed as [.., kc:kc+2]
        svec = consts.tile([P, EC + 1], FR, tag="svec")   # column sums of X
        g1c = consts.tile([P, EC + 1], FR, tag="g1c")     # scale*Wk^T bq
        g2c = consts.tile([P, EC + 1], FR, tag="g2c")     # G g1
        bqc = consts.tile([P, EC + 1], FR, tag="bqc")     # scale*bq column
        bkc = consts.tile([P, EC + 1], FR, tag="bkc")
        bvc = consts.tile([P, EC + 1], FR, tag="bvc")
        onec = consts.tile([P, 2], FR, tag="onec")
        u1row = consts.tile([1, E], FR, tag="u1row")
        u2row = consts.tile([1, E], FR, tag="u2row")
        v1row = consts.tile([1, E], FR, tag="v1row")
        v2row = consts.tile([1, E], FR, tag="v2row")
        borow = consts.tile([1, E], FR, tag="borow")
        crow = consts.tile([1, E], FR, tag="crow")
        tmpr0 = consts.tile([1, E], FR, tag="tmpr0")
        tmpr1 = consts.tile([1, E], FR, tag="tmpr1")
        alph = consts.tile([1, 1], F32, tag="alph")
        beta = consts.tile([1, 1], F32, tag="beta")
        absc = consts.tile([1, 1], F32, tag="absc")
        lA = consts.tile([P, E], FR, tag="lA")
        rA = consts.tile([P, E], FR, tag="rA")
        cpad = consts.tile([P, E], FR, tag="cpad")
        augone = consts.tile([P, P], FR, tag="augone")

        nc.sync.dma_start(onec[:], onein[:])
        nc.sync.dma_start(lA[:], zin[:])
        nc.sync.dma_start(rA[:], zin[:])
        nc.sync.dma_start(cpad[:], zin[:])
        nc.sync.dma_start(augone[:], augin[:])
        for tl in (svec, g1c, g2c, bqc, bkc, bvc):
            nc.sync.dma_start(tl[:], zin[:, :EC + 1])

        nc.sync.dma_start(bqc[:, :EC], bq.rearrange("(c p) -> p c", p=P))
        nc.sync.dma_start(bkc[:, :EC], bk.rearrange("(c p) -> p c", p=P))
        nc.sync.dma_start(bvc[:, :EC], bv.rearrange("(c p) -> p c", p=P))
        nc.sync.dma_start(borow[:], bo.rearrange("(a e) -> a e", a=1))

        xt_dram = dram.tile([E, SH], FR, tag="xt_dram", name="xt_dram")
        a_dram = dram.tile([E, E], FR, tag="a_dram", name="a_dram")

        # ---------------- Phase 1: XT = transpose(first SH rows of xb) ----
        with tc.tile_pool(name="xhp", bufs=2) as xhp:
            for so in range(SC):
                xhs = xhp.tile([P, E], FR, tag="xhs")
                nc.sync.dma_start(xhs[:], xb[so * P:(so + 1) * P, :])
                for ko in range(EC):
                    pt = pstr.tile([P, P], FR, tag="pt")
                    nc.tensor.transpose(pt[:], xhs[:, ko * P:(ko + 1) * P],
                                        ident[:])
                    st = stage.tile([P, P], FR, tag="xtst")
                    nc.scalar.copy(st[:], pt[:])
                    nc.sync.dma_start(
                        xt_dram[ko * P:(ko + 1) * P, so * P:(so + 1) * P],
                        st[:])

        # beta = bqs^T bk  (dot product; scale folded into bqc)
        pb = psv.tile([2, 2], F32, tag="psv")
        for kc in range(EC):
            mm(pb[:], bqc[:, kc:kc + 2], bkc[:, kc:kc + 2], kc == 0,
               kc == EC - 1)
        nc.vector.tensor_copy(beta[:], pb[0:1, 0:1])

        with tc.tile_pool(name="t1tp", bufs=1) as t1tp:
            with tc.tile_pool(name="gp", bufs=1) as gp:
                G = gp.tile([P, EC, E], FR, tag="G")

                # ---------- Phase 2: G = X^T X, svec = X^T 1 -------------
                with tc.tile_pool(name="xp", bufs=1) as xp:
                    X = xp.tile([P, KO, E], FR, tag="X")
                    for ko in range(KO):
                        nc.sync.dma_start(X[:, ko, :],
                                          xb[ko * P:(ko + 1) * P, :])
                    for mt in range(EC):
                        for nt in range(NT):
                            ps = psmm.tile([P, NF], F32, tag="psmm")
                            for ko in range(KO):
                                mm(ps[:], X[:, ko, mt * P:(mt + 1) * P],
                                   X[:, ko, nt * NF:(nt + 1) * NF],
                                   ko == 0, ko == KO - 1)
                            rcopy(G[:, mt, nt * NF:(nt + 1) * NF], ps[:])
                        pv = psv.tile([P, 2], F32, tag="psv")
                        for ko in range(KO):
                            mm(pv[:], X[:, ko, mt * P:(mt + 1) * P],
                               onec[:], ko == 0, ko == KO - 1)
                        rcopy(svec[:, mt:mt + 1], pv[:, 0:1])

                # ---------- Phase 3: Ut = Wk^T Wqs; u2row; g1c -----------
                with tc.tile_pool(name="utp", bufs=1) as utp:
                    UT = utp.tile([P, EC, E], FR, tag="UT")
                    with tc.tile_pool(name="wqp", bufs=1) as wqp:
                        WQ = wqp.tile([P, EC, E], FR, tag="WQ")
                        for kc in range(EC):
                            nc.sync.dma_start(WQ[:, kc, :],
                                              wq[kc * P:(kc + 1) * P, :])
                        with tc.tile_pool(name="wkp", bufs=2) as wkp:
                            for mt in range(EC):
                                WKm = wkp.tile([P, EC, P], FR, tag="WKm")
                                nc.sync.dma_start(
                                    WKm[:],
                                    wk.rearrange("(kc p) e -> p kc e", p=P)
                                    [:, :, mt * P:(mt + 1) * P])
                                for nt in range(NT):
                                    ps = psmm.tile([P, NF], F32, tag="psmm")
                                    for kc in range(EC):
                                        mm(ps[:], WKm[:, kc, :],
                                           WQ[:, kc, nt * NF:(nt + 1) * NF],
                                           kc == 0, kc == EC - 1)
                                    rcopy(UT[:, mt, nt * NF:(nt + 1) * NF],
                                          ps[:])
                                # g1c[mt] = (Wk^T bqs)[mt]
                                pv = psv.tile([P, 2], F32, tag="psv")
                                for kc in range(EC):
                                    mm(pv[:], WKm[:, kc, :], bqc[:, kc:kc + 2],
                                       kc == 0, kc == EC - 1)
                                rcopy(g1c[:, mt:mt + 1], pv[:, 0:1])
                        # u2row = bk^T Wqs  [1, E]
                        for nt in range(NT):
                            pr = psv.tile([2, NF], F32, tag="psv")
                            for kc in range(EC):
                                mm(pr[:], bkc[:, kc:kc + 2],
                                   WQ[:, kc, nt * NF:(nt + 1) * NF],
                                   kc == 0, kc == EC - 1)
                            rcopy(u2row[:, nt * NF:(nt + 1) * NF], pr[0:1, :])

                    # ---------- Phase 4: T1t = G^T Ut; u1row; g2c; alpha --
                    T1T = t1tp.tile([P, EC, E], FR, tag="T1T")
                    for mt in range(EC):
                        for nt in range(NT):
                            ps = psmm.tile([P, NF], F32, tag="psmm")
                            for kc in range(EC):
                                mm(ps[:], G[:, kc, mt * P:(mt + 1) * P],
                                   UT[:, kc, nt * NF:(nt + 1) * NF],
                                   kc == 0, kc == EC - 1)
                            rcopy(T1T[:, mt, nt * NF:(nt + 1) * NF], ps[:])
                    # u1row = s^T Ut
                    for nt in range(NT):
                        pr = psv.tile([2, NF], F32, tag="psv")
                        for kc in range(EC):
                            mm(pr[:], svec[:, kc:kc + 2],
                               UT[:, kc, nt * NF:(nt + 1) * NF],
                               kc == 0, kc == EC - 1)
                        rcopy(u1row[:, nt * NF:(nt + 1) * NF], pr[0:1, :])
                    # g2c = G g1 (G symmetric)
                    for mt in range(EC):
                        pv = psv.tile([P, 2], F32, tag="psv")
                        for kc in range(EC):
                            mm(pv[:], G[:, kc, mt * P:(mt + 1) * P],
                               g1c[:, kc:kc + 2], kc == 0, kc == EC - 1)
                        rcopy(g2c[:, mt:mt + 1], pv[:, 0:1])
                    # alpha = g1^T s
                    pa = psv.tile([2, 2], F32, tag="psv")
                    for kc in range(EC):
                        mm(pa[:], g1c[:, kc:kc + 2], svec[:, kc:kc + 2],
                           kc == 0, kc == EC - 1)
                    nc.vector.tensor_copy(alph[:], pa[0:1, 0:1])

            # ---------- Phase 5/6: WoT, Rt = Wv^T Wo^T; v1row ------------
            with tc.tile_pool(name="rtp", bufs=1) as rtp:
                RT = rtp.tile([P, EC, E], FR, tag="RT")
                with tc.tile_pool(name="wotp", bufs=1) as wotp:
                    WOT = wotp.tile([P, EC, E], FR, tag="WOT")
                    with tc.tile_pool(name="wop", bufs=2) as wop:
                        for fo in range(EC):
                            wos = wop.tile([P, E], FR, tag="wos")
                            nc.sync.dma_start(wos[:],
                                              wo[fo * P:(fo + 1) * P, :])
                            for kc in range(EC):
                                pt = pstr.tile([P, P], FR, tag="pt")
                                nc.tensor.transpose(
                                    pt[:], wos[:, kc * P:(kc + 1) * P],
                                    ident[:])
                                nc.scalar.copy(
                                    WOT[:, kc, fo * P:(fo + 1) * P], pt[:])
                    # v1row = bv^T Wo^T
                    for nt in range(NT):
                        pr = psv.tile([2, NF], F32, tag="psv")
                        for kc in range(EC):
                            mm(pr[:], bvc[:, kc:kc + 2],
                               WOT[:, kc, nt * NF:(nt + 1) * NF],
                               kc == 0, kc == EC - 1)
                        rcopy(v1row[:, nt * NF:(nt + 1) * NF], pr[0:1, :])
                    with tc.tile_pool(name="wvp", bufs=2) as wvp:
                        for mt in range(EC):
                            WVm = wvp.tile([P, EC, P], FR, tag="WVm")
                            nc.sync.dma_start(
                                WVm[:],
                                wv.rearrange("(kc p) e -> p kc e", p=P)
                                [:, :, mt * P:(mt + 1) * P])
                            for nt in range(NT):
                                ps = psmm.tile([P, NF], F32, tag="psmm")
                                for kc in range(EC):
                                    mm(ps[:], WVm[:, kc, :],
                                       WOT[:, kc, nt * NF:(nt + 1) * NF],
                                       kc == 0, kc == EC - 1)
                                rcopy(RT[:, mt, nt * NF:(nt + 1) * NF],
                                      ps[:])

                # ---------- Phase 7: rank-1 rows, A, c -------------------
                # v2row = s^T Rt ; crow(pre) = g2^T Rt
                for nt in range(NT):
                    pr = psv.tile([2, NF], F32, tag="psv")
                    for kc in range(EC):
                        mm(pr[:], svec[:, kc:kc + 2],
                           RT[:, kc, nt * NF:(nt + 1) * NF],
                           kc == 0, kc == EC - 1)
                    rcopy(v2row[:, nt * NF:(nt + 1) * NF], pr[0:1, :])
                for nt in range(NT):
                    pr = psv.tile([2, NF], F32, tag="psv")
                    for kc in range(EC):
                        mm(pr[:], g2c[:, kc:kc + 2],
                           RT[:, kc, nt * NF:(nt + 1) * NF],
                           kc == 0, kc == EC - 1)
                    rcopy(crow[:, nt * NF:(nt + 1) * NF], pr[0:1, :])

                # absc = alpha + S*beta ; crow += absc*v1row + beta*v2row + bo
                nc.vector.tensor_scalar(absc[:], beta[:], scaleS, alph[:],
                                        ALU.mult, ALU.add)
                nc.vector.tensor_scalar(tmpr0[:], v1row[:], absc[:1, :1],
                                        None, ALU.mult)
                nc.vector.tensor_tensor(crow[:], crow[:], tmpr0[:], ALU.add)
                nc.vector.tensor_scalar(tmpr0[:], v2row[:], beta[:1, :1],
                                        None, ALU.mult)
                nc.vector.tensor_tensor(crow[:], crow[:], tmpr0[:], ALU.add)
                nc.vector.tensor_tensor(crow[:], crow[:], borow[:], ALU.add)
                nc.sync.dma_start(cpad[0:1, :], crow[:])

                # lA rows: u1, u2 ; rA rows: v1, v2 + S*v1
                nc.sync.dma_start(lA[0:1, :], u1row[:])
                nc.sync.dma_start(lA[1:2, :], u2row[:])
                nc.sync.dma_start(rA[0:1, :], v1row[:])
                nc.vector.tensor_scalar(tmpr1[:], v1row[:], scaleS, None,
                                        ALU.mult)
                nc.vector.tensor_tensor(tmpr1[:], tmpr1[:], v2row[:], ALU.add)
                nc.sync.dma_start(rA[1:2, :], tmpr1[:])

                # A = T1t^T Rt + lA^T rA  -> a_dram
                for mt in range(EC):
                    for nt in range(NT):
                        ps = psmm.tile([P, NF], F32, tag="psmm")
                        for kc in range(EC):
                            mm(ps[:], T1T[:, kc, mt * P:(mt + 1) * P],
                               RT[:, kc, nt * NF:(nt + 1) * NF],
                               kc == 0, False)
                        mm(ps[:], lA[:, mt * P:(mt + 1) * P],
                           rA[:, nt * NF:(nt + 1) * NF], False, True)
                        ast = stage.tile([P, NF], FR, tag="ast")
                        rcopy(ast[:], ps[:])
                        nc.sync.dma_start(
                            a_dram[mt * P:(mt + 1) * P,
                                   nt * NF:(nt + 1) * NF], ast[:])

        # ---------------- Phase 8: Y = X_half A + 1 c^T ------------------
        with tc.tile_pool(name="yp", bufs=1) as yp:
            AF = yp.tile([P, EC, E], FR, tag="AF")
            for kc in range(EC):
                nc.sync.dma_start(AF[:, kc, :],
                                  a_dram[kc * P:(kc + 1) * P, :])
            with tc.tile_pool(name="xtp", bufs=2) as xtp:
                for mt in range(SC):
                    XTm = xtp.tile([P, EC, P], FR, tag="XTm")
                    nc.sync.dma_start(
                        XTm[:],
                        xt_dram[:].rearrange("(kc p) s -> p kc s", p=P)
                        [:, :, mt * P:(mt + 1) * P])
                    for nt in range(NT):
                        ps = psmm.tile([P, NF], F32, tag="psmm")
                        for kc in range(EC):
                            mm(ps[:], XTm[:, kc, :],
                               AF[:, kc, nt * NF:(nt + 1) * NF],
                               kc == 0, False)
                        mm(ps[:], augone[:], cpad[:, nt * NF:(nt + 1) * NF],
                           False, True)
                        yst = stage.tile([P, NF], F32, tag="yst")
                        nc.vector.tensor_copy(yst[:], ps[:])
                        nc.sync.dma_start(
                            y[mt * P:(mt + 1) * P, nt * NF:(nt + 1) * NF],
                            yst[:])


# ----------------------------------------------------------------------------
# Host side
# ----------------------------------------------------------------------------

def _rn22(a):
    """Round fp32 array to nearest fp22 (13 mantissa bits)."""
    a = np.ascontiguousarray(a, dtype=np.float32)
    b = a.view(np.uint32)
    return ((b + np.uint32(0x1000)) & np.uint32(0xFFFFE000)).view(np.float32)


_NC_CACHE = {}
RUN_KWARGS = {}       # test harness can set {"trace": True, "tmpdir": ...}
LAST_RESULTS = []     # BassKernelResults of each kernel() call


def _get_nc():
    key = "full"
    if key not in _NC_CACHE:
        _NC_CACHE[key] = build_nc(S=2048, SH=1024, E=1024, num_devices=8)
    return _NC_CACHE[key]


def kernel(x, Wq, bq, Wk, bk, Wv, bv, Wo, bo):
    from concourse.bass_utils import run_bass_kernel_spmd

    B, S, E = x.shape
    SH = S // 2
    SCALE = 0.125

    x = np.asarray(x, dtype=np.float32)
    wqs = _rn22(np.asarray(Wq, np.float32) * SCALE)
    bqs = _rn22(np.asarray(bq, np.float32) * SCALE)
    wkr = _rn22(Wk)
    wvr = _rn22(Wv)
    wor = _rn22(Wo)
    bkr = _rn22(bk)
    bvr = _rn22(bv)
    bof = np.asarray(bo, np.float32)

    aug128 = np.zeros((128, 128), dtype=np.float32)
    aug128[0, :] = 1.0
    in_maps = []
    for core in range(8):
        b, h = divmod(core, 2)
        xbp = x[b] if h == 0 else np.concatenate([x[b, SH:], x[b, :SH]], 0)
        in_maps.append({
            "xb": _rn22(xbp),
            "wq": wqs, "wk": wkr, "wv": wvr, "wo": wor,
            "bq": bqs, "bk": bkr, "bv": bvr, "bo": bof,
            "idin": np.eye(128, dtype=np.float32),
            "zin": np.zeros((128, E), dtype=np.float32),
            "augin": aug128,
            "onein": np.ones((128, 2), dtype=np.float32),
        })

    nc = _get_nc()
    res = run_bass_kernel_spmd(nc, in_maps, core_ids=list(range(8)),
                               **RUN_KWARGS)
    LAST_RESULTS.append(res)
    out = np.empty((B, S, E), dtype=np.float32)
    for core in range(8):
        b, h = divmod(core, 2)
        out[b, h * SH:(h + 1) * SH] = res.results[core]["y"]
    return out
